# revision 1
# baseline (speedup 1.0000x reference)
"""Trainium2 Bass kernel for nn_LinearKAN (histogram_binning).

Math
----
reference computes, per (batch b, out o):

    out[b,o] = sum_i  PL_interp(x[b,i]; bp[o,i,:], val[o,i,:])

where bp is the SAME sorted uniform grid for every (o,i) (tiled
linspace).  With u = (x - bp0)/h in [0, S), the piecewise-linear
interpolant has an exact *clamp basis* expansion

    f(u) = val_0 + sum_{s=0..S-1} M_s * clamp(u - s, 0, 1)
    M_s  = val_{s+1} - val_s              (segment slopes)

so the layer is a bias plus S dense matmuls contracting over (s, i):

    out[b,o] = bias[o] + sum_s sum_i M_s[o,i] * r_s[b,i]
    r_s      = clamp(u - s, 0, 1),   bias[o] = sum_i val[o,i,0]

The clamp basis quantizes benignly: r entries are exactly 0, exactly 1,
or the single fractional t per (b,i) -- so fp16 operands lose almost
nothing.  The slopes are split M = M_hi + M_lo/2048 with both parts
fp16 (2048 scaling keeps M_lo out of fp16-denormal range), accumulated
into two PSUM groups and combined in the tail:
out = ps_hi + 2^-11 * ps_lo + bias.  Measured ~2e-4 rel err.

Device kernel (per core, SPMD over 8 cores):
  - shard batch into 4 quarters (B_loc=256) x out-features into 2
    halves (O_loc=128); no cross-device reduction.
  - u^T [i, (ih,b)] via one ScalarE activation; r_s tiles [128, 512]
    fp16 via VectorE/ScalarE (relu then min-1); 2x40 fp16 matmuls
    (K=128 chunks of the (s,i) contraction) at full PE rate; tail
    DVE combine + bias; DMA out.
Host only slices/transposes/differences the params (layout prep).
"""

import os
import numpy as np

import concourse.bass as bass
import concourse.mybir as mybir
import concourse.tile as tile
from concourse import bacc
from concourse.bass_utils import run_bass_kernel_spmd

# Problem shape (hardcoded per the task contract).
B, O, I, S = 1024, 256, 256, 20
N_CORES = 8
B_SPLIT, O_SPLIT = 4, 2
B_LOC, O_LOC = B // B_SPLIT, O // O_SPLIT  # 256, 128
KT = 2 * S          # 40 K-tiles of 128 over the (s, i) contraction
CHUNK_KT = (4, 6, 8, 10, 12)  # C DMA chunk sizes in kt (smallest first)
LO_SCALE = 2048.0   # M_lo pre-scale (keeps fp16 normal); undone in tail
F32 = mybir.dt.float32
F16 = mybir.dt.float16
FW = 2 * B_LOC      # r/u tile free width: both i-halves side by side

# s values whose relu step runs on ScalarE (rest on VectorE); the min-1
# step always runs on VectorE.
ACT_RELU_S = set(range(8, 20))
N_WARMUP_MM = int(os.environ.get("KAN_WARMUP", "10"))  # PE HAM warmup dummies
N_GPS = int(os.environ.get("KAN_GPS", "0"))  # s-values built on GpSimd


def _strip_init_boilerplate(nc) -> None:
    """Drop the Bass-init const-AP memsets + all-engine barrier (~1.5us of
    preamble).  This kernel never reads the const APs (all activation biases
    are explicit APs), so the memsets and their barrier are dead weight."""
    blk = nc.m.functions[0].blocks[0]
    drop = (mybir.InstMemset, mybir.InstDrain, mybir.InstEventSemaphore)
    keep = [i for i in blk.instructions if not isinstance(i, drop)]
    del blk.instructions[:]
    for i in keep:
        blk.instructions.append(i)
    nc.const_aps.aps.clear()


def _build_nc(scale: float, ubias: float) -> bass.Bass:
    """Build the (SPMD-identical) single-core Bass graph."""
    nc = bacc.Bacc("TRN2", target_bir_lowering=False, debug=False)
    _strip_init_boilerplate(nc)

    xT = nc.declare_dram_parameter("xT", [128, FW], F32, isOutput=False)
    C2 = nc.declare_dram_parameter("C2", [128, 2 * KT * 128], F16,
                                   isOutput=False)
    bias0 = nc.declare_dram_parameter("bias0", [128, 1], F32, isOutput=False)
    out = nc.declare_dram_parameter("out", [O_LOC, B_LOC], F32, isOutput=True)

    with tile.TileContext(nc) as tc:
        with (
            tc.tile_pool(name="xt", bufs=1) as xpool,
            tc.tile_pool(name="u", bufs=1) as upool,
            tc.tile_pool(name="w", bufs=4) as wpool,
            tc.tile_pool(name="wact", bufs=len(ACT_RELU_S)) as wapool,
            tc.tile_pool(name="r", bufs=S) as rpool,
            tc.tile_pool(name="c", bufs=1) as cpool,
            tc.tile_pool(name="b", bufs=1) as bpool,
            tc.tile_pool(name="o", bufs=2) as opool,
            tc.tile_pool(name="ps", bufs=2, space="PSUM") as pspool,
        ):
            # --- ACT bias-constant table via gpsimd memsets (no DMA dep):
            # col 1 = ubias, col 1+s = -s for the ScalarE-assigned s.
            ctab = bpool.tile([128, 24], F32, tag="ctab")
            nc.gpsimd.memset(ctab[:, 1:2], float(ubias))
            for s in range(1, S):
                if s in ACT_RELU_S or s == S - 1:
                    nc.gpsimd.memset(ctab[:, 1 + s:2 + s], -float(s))

            # --- PE HAM warmup: dummy matmuls on memset scratch so the
            # clock-gate opens (1.2 -> 2.4 GHz) before the real stream.
            if N_WARMUP_MM:
                wa = wpool.tile([128, 128], F16, tag="warm_a")
                wb = wpool.tile([128, 512], F16, tag="warm_b")
                nc.gpsimd.memset(wa[:], 0.0)
                nc.gpsimd.memset(wb[:], 0.0)
                ps_warm = pspool.tile([128, 512], F32, tag="pw")
                for _ in range(N_WARMUP_MM):
                    nc.tensor.matmul(ps_warm[:], wa[:], wb[:],
                                     start=True, stop=True)

            # ACT table preload: cheap Copy on the memset const table.
            dummy = wpool.tile([128, 1], F32, tag="dummy")
            nc.scalar.copy(dummy[:], ctab[:, 1:2])

            # --- DMA in, all on the sync HWDGE queue; order matters:
            # xT first (it gates the whole ACT/DVE production chain),
            # then C chunks smallest-first; bias0 (tail-only) last.
            xt = xpool.tile([128, FW], F32)
            nc.sync.dma_start(xt[:], xT[:])
            chi = {}
            clo = {}
            kt0 = 0
            for ci, nkt in enumerate(CHUNK_KT):
                t = cpool.tile([128, nkt * 256], F16, tag=f"c{ci}")
                nc.sync.dma_start(
                    t[:], C2[:, kt0 * 256:(kt0 + nkt) * 256])
                for k in range(nkt):
                    chi[kt0 + k] = t[:, k * 128:(k + 1) * 128]
                    clo[kt0 + k] = t[:, (nkt + k) * 128:(nkt + k + 1) * 128]
                kt0 += nkt
            bias0_sb = bpool.tile([128, 1], F32, tag="b0")
            nc.sync.dma_start(bias0_sb[:], bias0[:])

            # --- u = relu(scale*x + ubias), one op over both i-halves ---
            u2 = upool.tile([128, FW], F32)
            nc.scalar.activation(
                u2[:], xt[:], mybir.ActivationFunctionType.Relu,
                bias=ctab[:, 1:2], scale=float(scale),
            )

            # --- r_s = clamp(u - s, 0, 1) in fp16 ---
            r = []
            for s in range(S):
                rs = rpool.tile([128, FW], F16, tag="r")
                if s == 0:
                    # u >= 0, so clamp(u,0,1) = min(u,1)
                    nc.vector.tensor_scalar(
                        rs[:], u2[:], 1.0, None, mybir.AluOpType.min)
                elif s == S - 1:
                    # u < 20, so clamp(u-19,0,1) = relu(u-19)
                    nc.scalar.activation(
                        rs[:], u2[:], mybir.ActivationFunctionType.Relu,
                        bias=ctab[:, 1 + s:2 + s], scale=1.0)
                elif s in ACT_RELU_S:
                    # fp16 intermediate: values >= 1 still clamp to exactly
                    # 1.0 after quantization, t-entries keep fp16 precision,
                    # and the 16-bit input speeds up the DVE min.
                    w = wapool.tile([128, FW], F16, tag="w_act")
                    nc.scalar.activation(
                        w[:], u2[:], mybir.ActivationFunctionType.Relu,
                        bias=ctab[:, 1 + s:2 + s], scale=1.0)
                    nc.vector.tensor_scalar(
                        rs[:], w[:], 1.0, None, mybir.AluOpType.min)
                elif s <= N_GPS:
                    w = wpool.tile([128, FW], F16, tag="w_gps")
                    nc.gpsimd.tensor_scalar(
                        w[:], u2[:], float(s), float(s),
                        mybir.AluOpType.max, mybir.AluOpType.subtract)
                    nc.gpsimd.tensor_scalar(
                        rs[:], w[:], 1.0, None, mybir.AluOpType.min)
                else:
                    w = wpool.tile([128, FW], F16, tag="w_dve")
                    nc.vector.tensor_scalar(
                        w[:], u2[:], float(s), float(s),
                        mybir.AluOpType.max, mybir.AluOpType.subtract)
                    nc.vector.tensor_scalar(
                        rs[:], w[:], 1.0, None, mybir.AluOpType.min)
                r.append(rs)

            # --- matmuls: hi/lo interleaved per kt, two PSUM groups ---
            ps_hi = pspool.tile([O_LOC, B_LOC], F32, tag="ph")
            ps_lo = pspool.tile([O_LOC, B_LOC], F32, tag="pl")
            if os.environ.get("KAN_INTERLEAVE", "1") == "1":
                for kt in range(KT):
                    s, ih = kt // 2, kt % 2
                    rhs = r[s][:, ih * B_LOC:(ih + 1) * B_LOC]
                    nc.tensor.matmul(ps_hi[:], chi[kt], rhs,
                                     start=(kt == 0), stop=(kt == KT - 1))
                    nc.tensor.matmul(ps_lo[:], clo[kt], rhs,
                                     start=(kt == 0), stop=(kt == KT - 1))
            else:
                for ps, carr in ((ps_hi, chi), (ps_lo, clo)):
                    for kt in range(KT):
                        s, ih = kt // 2, kt % 2
                        rhs = r[s][:, ih * B_LOC:(ih + 1) * B_LOC]
                        nc.tensor.matmul(ps[:], carr[kt], rhs,
                                         start=(kt == 0), stop=(kt == KT - 1))

            # --- tail: out = ps_hi + ps_lo/2048 + bias ---
            t1 = opool.tile([O_LOC, B_LOC], F32, tag="t1")
            nc.vector.tensor_scalar(
                t1[:], ps_lo[:], 1.0 / LO_SCALE, bias0_sb[:, 0:1],
                mybir.AluOpType.mult, mybir.AluOpType.add)
            out_sb = opool.tile([O_LOC, B_LOC], F32, tag="osb")
            nc.vector.tensor_tensor(
                out_sb[:], ps_hi[:], t1[:], mybir.AluOpType.add)
            nc.sync.dma_start(out[:], out_sb[:])
    nc.compile()
    return nc


_NC_CACHE: dict = {}


def _get_nc(scale: float, ubias: float) -> bass.Bass:
    key = (float(scale), float(ubias))
    if key not in _NC_CACHE:
        _NC_CACHE[key] = _build_nc(scale, ubias)
    return _NC_CACHE[key]


def prepare(x: np.ndarray, breakpoints: np.ndarray, values: np.ndarray):
    """Host prep: build the Bass graph (cached) + per-core input maps."""
    x = np.asarray(x, np.float32)
    breakpoints = np.asarray(breakpoints, np.float32)
    values = np.asarray(values, np.float32)

    # Grid affine params from the (shared) breakpoint row.
    bpr = breakpoints[0, 0].astype(np.float64)
    h = (bpr[-1] - bpr[0]) / S
    scale = float(1.0 / h)
    ubias = float(-bpr[0] / h)

    # Clamp-basis slopes, split into fp16 hi + scaled fp16 lo.
    Vf = values  # [O, I, S+1]
    M = (Vf[:, :, 1:] - Vf[:, :, :-1]).transpose(2, 0, 1)  # [S, O, I] f32
    M = np.ascontiguousarray(M, np.float32)
    Mhi = M.astype(np.float16)
    Mlo = ((M - Mhi.astype(np.float32)) * LO_SCALE).astype(np.float16)
    bias_o = Vf[:, :, 0].sum(axis=1, dtype=np.float64).astype(np.float32)

    # Per-core layouts.
    #   C*: [j, kt, o] fp16 with kt = 2*s + ih, j = i within half.
    #   xT: [j, ih*B_LOC + b] fp32.
    Mhi_r = Mhi.reshape(S, O_SPLIT, O_LOC, 2, 128)  # [s, oh, o, ih, j]
    Mlo_r = Mlo.reshape(S, O_SPLIT, O_LOC, 2, 128)
    xr = x.reshape(B_SPLIT, B_LOC, 2, 128)          # [bq, b, ih, j]

    in_maps = []
    for c in range(N_CORES):
        bq, oh = c % B_SPLIT, c // B_SPLIT
        # xr[bq] axes (b, ih, j) -> (j, ih, b) -> [128, FW]
        xT_c = np.ascontiguousarray(
            xr[bq].transpose(2, 1, 0)).reshape(128, FW)
        C_hi = np.ascontiguousarray(
            Mhi_r[:, oh].transpose(3, 0, 2, 1)).reshape(128, KT * 128)
        C_lo = np.ascontiguousarray(
            Mlo_r[:, oh].transpose(3, 0, 2, 1)).reshape(128, KT * 128)
        # Interleave hi/lo per DMA chunk: [hi kts of chunk][lo kts of chunk]
        blocks = []
        kt0 = 0
        for nkt in CHUNK_KT:
            blocks.append(C_hi[:, kt0 * 128:(kt0 + nkt) * 128])
            blocks.append(C_lo[:, kt0 * 128:(kt0 + nkt) * 128])
            kt0 += nkt
        C2_c = np.ascontiguousarray(np.concatenate(blocks, axis=1))
        b0 = np.ascontiguousarray(
            bias_o[oh * O_LOC:(oh + 1) * O_LOC].reshape(128, 1))
        in_maps.append({"xT": xT_c, "C2": C2_c, "bias0": b0})

    nc = _get_nc(scale, ubias)
    return nc, in_maps


def kernel(x: np.ndarray, breakpoints: np.ndarray, values: np.ndarray,
           **_extra) -> np.ndarray:
    nc, in_maps = prepare(x, breakpoints, values)
    res = run_bass_kernel_spmd(nc, in_maps, list(range(N_CORES)))

    outf = np.empty((B, O), np.float32)
    for c in range(N_CORES):
        bq, oh = c % B_SPLIT, c // B_SPLIT
        outf[bq * B_LOC:(bq + 1) * B_LOC, oh * O_LOC:(oh + 1) * O_LOC] = \
            res.results[c]["out"].T
    return outf


if __name__ == "__main__":
    rng = np.random.default_rng(0)
    x = rng.uniform(-1, 1, (B, I)).astype(np.float32)
    bp = np.tile(np.linspace(-1, 1, S + 1, dtype=np.float32), (O, I, 1))
    v = (rng.standard_normal((O, I, S + 1)) * 0.1).astype(np.float32)
    out = kernel(x, bp, v)
    print("kernel ran, out:", out.shape, out.dtype, float(out.std()))



# revision 5
# speedup vs baseline: 1.1990x; 1.1990x over previous
"""Trainium2 Bass kernel for nn_LinearKAN (histogram_binning).

Math
----
reference computes, per (batch b, out o):

    out[b,o] = sum_i  PL_interp(x[b,i]; bp[o,i,:], val[o,i,:])

where bp is the SAME sorted uniform grid for every (o,i) (tiled
linspace).  With u = (x - bp0)/h in [0, S), the piecewise-linear
interpolant has an exact *clamp basis* expansion

    f(u) = val_0 + sum_{s=0..S-1} M_s * clamp(u - s, 0, 1)
    M_s  = val_{s+1} - val_s              (segment slopes)

so the layer is a bias plus S dense matmuls contracting over (s, i).

Device mapping (v2, single fp16 stream):
  - One-instruction clamp: g_s = min(max(u_q, a), a+1) with a = s%4 and
    u_q = u - 4*(s//4).  The block shift keeps every fp16 saturation an
    exact small integer, and the "- a" offset folds into the bias:
    bias_o = sum_i val0[o,i] - sum_s (s%4) * sum_i M_s[o,i].
  - u0 = (x - bp0)/h is precomputed on HOST in f64 and shipped as fp16
    (128 KB/core instead of 256 KB fp32 x); u_q tiles are exact fp16
    subtractions of 4q.  End-to-end rel err ~5.7e-3 (numpy-simulated),
    well under the 2e-2 gate.
  - M fp16 single stream (no hi/lo split): 40 K=128 matmuls per core,
    C traffic 1.31 MB/core.
  - bias is seeded INTO PSUM by two K=1 matmuls (bias_hi x ones,
    bias_lo*2048 x ones*2^-11) during the PE warmup window, so the tail
    is just one ACT copy PSUM->SBUF + DMA out.
  - shard: batch in 4 quarters (B_loc=256) x out-features in 2 halves
    (O_loc=128) over 8 cores; no cross-device reduction.
"""

import os
import numpy as np

import concourse.bass as bass
import concourse.mybir as mybir
import concourse.tile as tile
from concourse import bacc
from concourse.bass_utils import run_bass_kernel_spmd

# Problem shape (hardcoded per the task contract).
B, O, I, S = 1024, 256, 256, 20
N_CORES = 8
B_SPLIT, O_SPLIT = 4, 2
B_LOC, O_LOC = B // B_SPLIT, O // O_SPLIT  # 256, 128
KT = 2 * S          # 40 K-tiles of 128 over the (s, i) contraction
F32 = mybir.dt.float32
F16 = mybir.dt.float16
FW = 2 * B_LOC      # free width of u/g tiles: both i-halves side by side
QW = 4              # u-block width: u_q = u - 4q, q = 0..4

LO_ONES = 2.0 ** -11   # seed-matmul rhs for the bias lo part
LO_SCALE = 2048.0      # host pre-scale of bias lo (keeps fp16 normal)


def _envtuple(name, default):
    v = os.environ.get(name)
    if not v:
        return default
    return tuple(int(t) for t in v.split(",") if t != "")


# --- tunables (env-overridable for perf iteration) ---
N_WARMUP_MM = int(os.environ.get("KAN_WARMUP", "4"))  # PE clock-warmup mms
WARM_N = int(os.environ.get("KAN_WARM_N", "512"))     # warmup rhs width
CHUNK_KT = _envtuple("KAN_CHUNKS", (4, 8, 12, 16))    # C DMA chunk sizes
# Pool (gpsimd) tensor_scalar is a ~7.5us software loop on Q7 -- never
# put g ops there (measured).  ACT can do relu-only s (top segment) and
# the u_q Copy-with-bias tiles.
POOL_S = set(_envtuple("KAN_POOL_S", ()))             # g ops on Pool engine
ACT_S = set(_envtuple("KAN_ACT_S", (19,)))            # relu-only s on ACT
ACT_UQ = set(_envtuple("KAN_ACT_UQ", (3, 4)))         # u_q built on ACT


def _strip_init_boilerplate(nc) -> None:
    """Drop the Bass-init const-AP memsets + all-engine barrier (~1.5us of
    preamble).  All activation biases here are explicit APs or float biases
    on Copy, so the const-AP memsets and their barrier are dead weight."""
    blk = nc.m.functions[0].blocks[0]
    drop = (mybir.InstMemset, mybir.InstDrain, mybir.InstEventSemaphore)
    keep = [i for i in blk.instructions if not isinstance(i, drop)]
    del blk.instructions[:]
    for i in keep:
        blk.instructions.append(i)
    nc.const_aps.aps.clear()


def _build_nc() -> bass.Bass:
    """Build the (SPMD-identical) single-core Bass graph."""
    assert sum(CHUNK_KT) == KT, CHUNK_KT
    nc = bacc.Bacc("TRN2", target_bir_lowering=False, debug=False)
    _strip_init_boilerplate(nc)

    u0d = nc.declare_dram_parameter("u0", [128, FW], F16, isOutput=False)
    Cd = nc.declare_dram_parameter("C", [128, KT * 128], F16, isOutput=False)
    b2d = nc.declare_dram_parameter("b2", [1, 256], F16, isOutput=False)
    out = nc.declare_dram_parameter("out", [O_LOC, B_LOC], F32, isOutput=True)

    with tile.TileContext(nc) as tc:
        with (
            tc.tile_pool(name="u", bufs=6) as upool,
            tc.tile_pool(name="g", bufs=S) as gpool,
            tc.tile_pool(name="c", bufs=len(CHUNK_KT)) as cpool,
            tc.tile_pool(name="w", bufs=4) as wpool,
            tc.tile_pool(name="o", bufs=1) as opool,
            tc.tile_pool(name="ps", bufs=2, space="PSUM") as pspool,
        ):
            # --- Pool-engine memsets (no DMA dep): warmup operands, the
            # two seed rows, and the ACT bias column for the relu-only s.
            ones_hi = wpool.tile([1, B_LOC], F16, tag="ones_hi")
            ones_lo = wpool.tile([1, B_LOC], F16, tag="ones_lo")
            nc.gpsimd.memset(ones_hi[:], 1.0)
            nc.gpsimd.memset(ones_lo[:], LO_ONES)
            actb = wpool.tile([128, len(ACT_S) or 1], F32, tag="actb")
            for k, s in enumerate(sorted(ACT_S)):
                nc.gpsimd.memset(actb[:, k:k + 1], -float(s))

            # --- PE HAM warmup: dummy matmuls on memset scratch so the
            # clock-gate opens (1.2 -> 2.4 GHz) before the real stream.
            if N_WARMUP_MM:
                wa = wpool.tile([128, 128], F16, tag="warm_a")
                wb = wpool.tile([128, WARM_N], F16, tag="warm_b")
                nc.gpsimd.memset(wa[:], 0.0)
                nc.gpsimd.memset(wb[:], 0.0)
                ps_warm = pspool.tile([128, WARM_N], F32, tag="pw")
                for _ in range(N_WARMUP_MM):
                    nc.tensor.matmul(ps_warm[:], wa[:], wb[:],
                                     start=True, stop=True)

            # --- DMA in (sync HWDGE): u0 first (gates the whole
            # elementwise chain), tiny bias row next (gates the psum
            # seed), then C chunks smallest-first.
            u0 = upool.tile([128, FW], F16, tag="u0")
            nc.sync.dma_start(u0[:], u0d[:])
            b2 = wpool.tile([1, 256], F16, tag="b2")
            nc.sync.dma_start(b2[:], b2d[:])
            ckt = {}
            kt0 = 0
            for ci, nkt in enumerate(CHUNK_KT):
                t = cpool.tile([128, nkt * 128], F16, tag=f"c{ci}")
                nc.sync.dma_start(t[:], Cd[:, kt0 * 128:(kt0 + nkt) * 128])
                for k in range(nkt):
                    ckt[kt0 + k] = t[:, k * 128:(k + 1) * 128]
                kt0 += nkt

            # --- PSUM bias seed: ps = bias_hi + bias_lo*2048 * 2^-11 ---
            ps = pspool.tile([O_LOC, B_LOC], F32, tag="ps")
            nc.tensor.matmul(ps[:], b2[:, 0:O_LOC], ones_hi[:],
                             start=True, stop=False, skip_group_check=True)
            nc.tensor.matmul(ps[:], b2[:, O_LOC:2 * O_LOC], ones_lo[:],
                             start=False, stop=False, skip_group_check=True)

            # --- u_q = u0 - 4q (fp16-exact for every value that matters).
            # Early blocks on DVE (needed soonest), late ones on the
            # otherwise-idle ACT engine (Copy with float bias).
            uq = {0: u0}
            for q in range(1, (S + QW - 1) // QW):
                t = upool.tile([128, FW], F16, tag="uq")
                if q in ACT_UQ:
                    nc.scalar.activation(
                        t[:], u0[:], mybir.ActivationFunctionType.Copy,
                        bias=-float(QW * q), scale=1.0)
                else:
                    nc.vector.tensor_scalar(
                        t[:], u0[:], float(QW * q), None,
                        mybir.AluOpType.subtract)
                uq[q] = t

            # --- g_s = min(max(u_q, a), a+1), a = s%4; one op each ---
            g = []
            for s in range(S):
                gs = gpool.tile([128, FW], F16, tag="g")
                q, a = s // QW, float(s % QW)
                if s in ACT_S:
                    # u < S strictly, so clamp(u-s,0,1) = relu(u-s) for
                    # the top segment; ACT engine is otherwise idle.
                    k = sorted(ACT_S).index(s)
                    nc.scalar.activation(
                        gs[:], u0[:], mybir.ActivationFunctionType.Relu,
                        bias=actb[:, k:k + 1], scale=1.0)
                elif s in POOL_S:
                    nc.gpsimd.tensor_scalar(
                        gs[:], uq[q][:], a, a + 1.0,
                        mybir.AluOpType.max, mybir.AluOpType.min)
                else:
                    nc.vector.tensor_scalar(
                        gs[:], uq[q][:], a, a + 1.0,
                        mybir.AluOpType.max, mybir.AluOpType.min)
                g.append(gs)

            # --- 40 accumulating matmuls over kt = (s, ih) ---
            for kt in range(KT):
                s, ih = kt // 2, kt % 2
                rhs = g[s][:, ih * B_LOC:(ih + 1) * B_LOC]
                nc.tensor.matmul(ps[:], ckt[kt], rhs,
                                 start=False, stop=(kt == KT - 1),
                                 skip_group_check=True)

            # --- tail: one ACT copy PSUM -> SBUF, then DMA out ---
            out_sb = opool.tile([O_LOC, B_LOC], F32, tag="osb")
            nc.scalar.copy(out_sb[:], ps[:])
            nc.sync.dma_start(out[:], out_sb[:])
    nc.compile()
    return nc


_NC_CACHE: dict = {}


def _get_nc() -> bass.Bass:
    if "nc" not in _NC_CACHE:
        _NC_CACHE["nc"] = _build_nc()
    return _NC_CACHE["nc"]


def prepare(x: np.ndarray, breakpoints: np.ndarray, values: np.ndarray):
    """Host prep: build the Bass graph (cached) + per-core input maps."""
    x = np.asarray(x)
    values = np.asarray(values)

    # Grid affine params from the (shared) breakpoint row.
    bpr = np.asarray(breakpoints)[0, 0].astype(np.float64)
    h = (bpr[-1] - bpr[0]) / S
    scale = 1.0 / h
    ubias = -float(bpr[0]) / h

    # u in [0, S) computed on host in f64, shipped fp16.
    u = (x.astype(np.float64) * scale + ubias)
    u16 = u.astype(np.float16)

    # Clamp-basis slopes (fp16) and the folded bias (f64 -> hi/lo fp16).
    Vf = values.astype(np.float64)          # [O, I, S+1]
    M = (Vf[:, :, 1:] - Vf[:, :, :-1]).transpose(2, 0, 1)  # [S, O, I]
    M16 = M.astype(np.float16)
    # ACT-assigned s produce the UNshifted clamp (relu), so their fold
    # offset is 0, not s%4.
    amod = np.array([0.0 if s in ACT_S else float(s % QW) for s in range(S)])
    bias_o = Vf[:, :, 0].sum(axis=1) - np.einsum(
        "s,soi->o", amod, M, optimize=True)   # [O] f64
    bh = bias_o.astype(np.float16)
    bl = ((bias_o - bh.astype(np.float64)) * LO_SCALE).astype(np.float16)

    # Per-core layouts.
    M16_r = M16.reshape(S, O_SPLIT, O_LOC, 2, 128)  # [s, oh, o, ih, j]
    ur = u16.reshape(B_SPLIT, B_LOC, 2, 128)        # [bq, b, ih, j]

    in_maps = []
    for c in range(N_CORES):
        bq, oh = c % B_SPLIT, c // B_SPLIT
        # ur[bq] axes (b, ih, j) -> (j, ih, b) -> [128, FW]
        u0_c = np.ascontiguousarray(
            ur[bq].transpose(2, 1, 0)).reshape(128, FW)
        # [s, o, ih, j] -> (j, s, ih, o): columns kt*128 + o, kt = 2s+ih
        C_c = np.ascontiguousarray(
            M16_r[:, oh].transpose(3, 0, 2, 1)).reshape(128, KT * 128)
        b2_c = np.ascontiguousarray(np.concatenate(
            [bh[oh * O_LOC:(oh + 1) * O_LOC],
             bl[oh * O_LOC:(oh + 1) * O_LOC]]).reshape(1, 256))
        in_maps.append({"u0": u0_c, "C": C_c, "b2": b2_c})

    nc = _get_nc()
    return nc, in_maps


def kernel(x: np.ndarray, breakpoints: np.ndarray, values: np.ndarray,
           **_extra) -> np.ndarray:
    nc, in_maps = prepare(x, breakpoints, values)
    res = run_bass_kernel_spmd(nc, in_maps, list(range(N_CORES)))

    outf = np.empty((B, O), np.float32)
    for c in range(N_CORES):
        bq, oh = c % B_SPLIT, c // B_SPLIT
        outf[bq * B_LOC:(bq + 1) * B_LOC, oh * O_LOC:(oh + 1) * O_LOC] = \
            res.results[c]["out"].T
    return outf


if __name__ == "__main__":
    rng = np.random.default_rng(0)
    x = rng.uniform(-1, 1, (B, I)).astype(np.float32)
    bp = np.tile(np.linspace(-1, 1, S + 1, dtype=np.float32), (O, I, 1))
    v = (rng.standard_normal((O, I, S + 1)) * 0.1).astype(np.float32)
    out = kernel(x, bp, v)
    print("kernel ran, out:", out.shape, out.dtype, float(out.std()))


# revision 6
# speedup vs baseline: 1.2328x; 1.0282x over previous
"""Trainium2 Bass kernel for nn_LinearKAN (histogram_binning).

Math
----
reference computes, per (batch b, out o):

    out[b,o] = sum_i  PL_interp(x[b,i]; bp[o,i,:], val[o,i,:])

where bp is the SAME sorted uniform grid for every (o,i) (tiled
linspace).  With u = (x - bp0)/h in [0, S), the piecewise-linear
interpolant has an exact *clamp basis* expansion

    f(u) = val_0 + sum_{s=0..S-1} M_s * clamp(u - s, 0, 1)
    M_s  = val_{s+1} - val_s              (segment slopes)

so the layer is a bias plus S dense matmuls contracting over (s, i).

Device mapping (v2, single fp16 stream):
  - One-instruction clamp: g_s = min(max(u_q, a), a+1) with a = s%4 and
    u_q = u - 4*(s//4).  The block shift keeps every fp16 saturation an
    exact small integer, and the "- a" offset folds into the bias:
    bias_o = sum_i val0[o,i] - sum_s (s%4) * sum_i M_s[o,i].
  - u0 = (x - bp0)/h is precomputed on HOST in f64 and shipped as fp16
    (128 KB/core instead of 256 KB fp32 x); u_q tiles are exact fp16
    subtractions of 4q.  End-to-end rel err ~5.7e-3 (numpy-simulated),
    well under the 2e-2 gate.
  - M fp16 single stream (no hi/lo split): 40 K=128 matmuls per core,
    C traffic 1.31 MB/core.
  - bias is seeded INTO PSUM by two K=1 matmuls (bias_hi x ones,
    bias_lo*2048 x ones*2^-11) during the PE warmup window, so the tail
    is just one ACT copy PSUM->SBUF + DMA out.
  - shard: batch in 4 quarters (B_loc=256) x out-features in 2 halves
    (O_loc=128) over 8 cores; no cross-device reduction.
"""

import os
import numpy as np

import concourse.bass as bass
import concourse.mybir as mybir
import concourse.tile as tile
from concourse import bacc
from concourse.bass_utils import run_bass_kernel_spmd

# Problem shape (hardcoded per the task contract).
B, O, I, S = 1024, 256, 256, 20
N_CORES = 8
B_SPLIT, O_SPLIT = 4, 2
B_LOC, O_LOC = B // B_SPLIT, O // O_SPLIT  # 256, 128
KT = 2 * S          # 40 K-tiles of 128 over the (s, i) contraction
F32 = mybir.dt.float32
F16 = mybir.dt.float16
FW = 2 * B_LOC      # free width of u/g tiles: both i-halves side by side
QW = 4              # u-block width: u_q = u - 4q, q = 0..4

LO_ONES = 2.0 ** -11   # seed-matmul rhs for the bias lo part
LO_SCALE = 2048.0      # host pre-scale of bias lo (keeps fp16 normal)


def _envtuple(name, default):
    v = os.environ.get(name)
    if not v:
        return default
    return tuple(int(t) for t in v.split(",") if t != "")


# --- tunables (env-overridable for perf iteration) ---
N_WARMUP_MM = int(os.environ.get("KAN_WARMUP", "10"))  # PE clock-warmup mms
WARM_N = int(os.environ.get("KAN_WARM_N", "256"))     # warmup rhs width
CHUNK_KT = _envtuple("KAN_CHUNKS", (4, 12, 12, 12))   # C DMA chunk sizes
# Pool (gpsimd) tensor_scalar is a ~7.5us software loop on Q7 -- never
# put g ops there (measured).  ACT can do relu-only s (top segment) and
# the u_q Copy-with-bias tiles.
POOL_S = set(_envtuple("KAN_POOL_S", ()))             # g ops on Pool engine
ACT_S = set(_envtuple("KAN_ACT_S", (19,)))            # relu-only s on ACT
ACT_UQ = set(_envtuple("KAN_ACT_UQ", (3, 4)))         # u_q built on ACT


def _strip_init_boilerplate(nc) -> None:
    """Drop the Bass-init const-AP memsets + all-engine barrier (~1.5us of
    preamble).  All activation biases here are explicit APs or float biases
    on Copy, so the const-AP memsets and their barrier are dead weight."""
    blk = nc.m.functions[0].blocks[0]
    drop = (mybir.InstMemset, mybir.InstDrain, mybir.InstEventSemaphore)
    keep = [i for i in blk.instructions if not isinstance(i, drop)]
    del blk.instructions[:]
    for i in keep:
        blk.instructions.append(i)
    nc.const_aps.aps.clear()


def _build_nc() -> bass.Bass:
    """Build the (SPMD-identical) single-core Bass graph."""
    assert sum(CHUNK_KT) == KT, CHUNK_KT
    nc = bacc.Bacc("TRN2", target_bir_lowering=False, debug=False)
    _strip_init_boilerplate(nc)

    u0d = nc.declare_dram_parameter("u0", [128, FW], F16, isOutput=False)
    Cd = nc.declare_dram_parameter("C", [128, KT * 128], F16, isOutput=False)
    b2d = nc.declare_dram_parameter("b2", [1, 256], F16, isOutput=False)
    out = nc.declare_dram_parameter("out", [O_LOC, B_LOC], F16, isOutput=True)

    with tile.TileContext(nc) as tc:
        with (
            tc.tile_pool(name="u", bufs=6) as upool,
            tc.tile_pool(name="g", bufs=S) as gpool,
            tc.tile_pool(name="c", bufs=len(CHUNK_KT)) as cpool,
            tc.tile_pool(name="w", bufs=4) as wpool,
            tc.tile_pool(name="o", bufs=1) as opool,
            tc.tile_pool(name="ps", bufs=2, space="PSUM") as pspool,
        ):
            # --- Pool-engine memsets (no DMA dep): warmup operands, the
            # two seed rows, and the ACT bias column for the relu-only s.
            ones_hi = wpool.tile([1, B_LOC], F16, tag="ones_hi")
            ones_lo = wpool.tile([1, B_LOC], F16, tag="ones_lo")
            nc.gpsimd.memset(ones_hi[:], 1.0)
            nc.gpsimd.memset(ones_lo[:], LO_ONES)
            actb = wpool.tile([128, len(ACT_S) or 1], F32, tag="actb")
            for k, s in enumerate(sorted(ACT_S)):
                nc.gpsimd.memset(actb[:, k:k + 1], -float(s))

            # --- PE HAM warmup: dummy matmuls on memset scratch so the
            # clock-gate opens (1.2 -> 2.4 GHz) before the real stream.
            if N_WARMUP_MM:
                wa = wpool.tile([128, 128], F16, tag="warm_a")
                wb = wpool.tile([128, WARM_N], F16, tag="warm_b")
                nc.gpsimd.memset(wa[:], 0.0)
                nc.gpsimd.memset(wb[:], 0.0)
                ps_warm = pspool.tile([128, WARM_N], F32, tag="pw")
                for _ in range(N_WARMUP_MM):
                    nc.tensor.matmul(ps_warm[:], wa[:], wb[:],
                                     start=True, stop=True)

            # --- DMA in (sync HWDGE): u0 first (gates the whole
            # elementwise chain), tiny bias row next (gates the psum
            # seed), then C chunks smallest-first.
            u0 = upool.tile([128, FW], F16, tag="u0")
            nc.sync.dma_start(u0[:], u0d[:])
            b2 = wpool.tile([1, 256], F16, tag="b2")
            ckt = {}
            kt0 = 0
            for ci, nkt in enumerate(CHUNK_KT):
                t = cpool.tile([128, nkt * 128], F16, tag=f"c{ci}")
                nc.sync.dma_start(t[:], Cd[:, kt0 * 128:(kt0 + nkt) * 128])
                for k in range(nkt):
                    ckt[kt0 + k] = t[:, k * 128:(k + 1) * 128]
                kt0 += nkt
                if ci == 0:
                    # tiny bias row slots in right after the first chunk
                    nc.sync.dma_start(b2[:], b2d[:])

            # --- PSUM bias seed: ps = bias_hi + bias_lo*2048 * 2^-11 ---
            ps = pspool.tile([O_LOC, B_LOC], F32, tag="ps")
            nc.tensor.matmul(ps[:], b2[:, 0:O_LOC], ones_hi[:],
                             start=True, stop=False, skip_group_check=True)
            nc.tensor.matmul(ps[:], b2[:, O_LOC:2 * O_LOC], ones_lo[:],
                             start=False, stop=False, skip_group_check=True)

            # --- u_q = u0 - 4q (fp16-exact for every value that matters).
            # Early blocks on DVE (needed soonest), late ones on the
            # otherwise-idle ACT engine (Copy with float bias).
            uq = {0: u0}
            for q in range(1, (S + QW - 1) // QW):
                t = upool.tile([128, FW], F16, tag="uq")
                if q in ACT_UQ:
                    nc.scalar.activation(
                        t[:], u0[:], mybir.ActivationFunctionType.Copy,
                        bias=-float(QW * q), scale=1.0)
                else:
                    nc.vector.tensor_scalar(
                        t[:], u0[:], float(QW * q), None,
                        mybir.AluOpType.subtract)
                uq[q] = t

            # --- g_s = min(max(u_q, a), a+1), a = s%4; one op each ---
            g = []
            for s in range(S):
                gs = gpool.tile([128, FW], F16, tag="g")
                q, a = s // QW, float(s % QW)
                if s in ACT_S:
                    # u < S strictly, so clamp(u-s,0,1) = relu(u-s) for
                    # the top segment; ACT engine is otherwise idle.
                    k = sorted(ACT_S).index(s)
                    nc.scalar.activation(
                        gs[:], u0[:], mybir.ActivationFunctionType.Relu,
                        bias=actb[:, k:k + 1], scale=1.0)
                elif s in POOL_S:
                    nc.gpsimd.tensor_scalar(
                        gs[:], uq[q][:], a, a + 1.0,
                        mybir.AluOpType.max, mybir.AluOpType.min)
                else:
                    nc.vector.tensor_scalar(
                        gs[:], uq[q][:], a, a + 1.0,
                        mybir.AluOpType.max, mybir.AluOpType.min)
                g.append(gs)

            # --- 40 accumulating matmuls over kt = (s, ih) ---
            for kt in range(KT):
                s, ih = kt // 2, kt % 2
                rhs = g[s][:, ih * B_LOC:(ih + 1) * B_LOC]
                nc.tensor.matmul(ps[:], ckt[kt], rhs,
                                 start=False, stop=(kt == KT - 1),
                                 skip_group_check=True)

            # --- tail: one ACT copy PSUM -> SBUF, then DMA out ---
            out_sb = opool.tile([O_LOC, B_LOC], F16, tag="osb")
            nc.scalar.copy(out_sb[:], ps[:])
            nc.sync.dma_start(out[:], out_sb[:])
    nc.compile()
    return nc


_NC_CACHE: dict = {}


def _get_nc() -> bass.Bass:
    if "nc" not in _NC_CACHE:
        _NC_CACHE["nc"] = _build_nc()
    return _NC_CACHE["nc"]


def prepare(x: np.ndarray, breakpoints: np.ndarray, values: np.ndarray):
    """Host prep: build the Bass graph (cached) + per-core input maps."""
    x = np.asarray(x)
    values = np.asarray(values)

    # Grid affine params from the (shared) breakpoint row.
    bpr = np.asarray(breakpoints)[0, 0].astype(np.float64)
    h = (bpr[-1] - bpr[0]) / S
    scale = 1.0 / h
    ubias = -float(bpr[0]) / h

    # u in [0, S) computed on host in f64, shipped fp16.
    u = (x.astype(np.float64) * scale + ubias)
    u16 = u.astype(np.float16)

    # Clamp-basis slopes (fp16) and the folded bias (f64 -> hi/lo fp16).
    Vf = values.astype(np.float64)          # [O, I, S+1]
    M = (Vf[:, :, 1:] - Vf[:, :, :-1]).transpose(2, 0, 1)  # [S, O, I]
    M16 = M.astype(np.float16)
    # ACT-assigned s produce the UNshifted clamp (relu), so their fold
    # offset is 0, not s%4.
    amod = np.array([0.0 if s in ACT_S else float(s % QW) for s in range(S)])
    bias_o = Vf[:, :, 0].sum(axis=1) - np.einsum(
        "s,soi->o", amod, M, optimize=True)   # [O] f64
    bh = bias_o.astype(np.float16)
    bl = ((bias_o - bh.astype(np.float64)) * LO_SCALE).astype(np.float16)

    # Per-core layouts.
    M16_r = M16.reshape(S, O_SPLIT, O_LOC, 2, 128)  # [s, oh, o, ih, j]
    ur = u16.reshape(B_SPLIT, B_LOC, 2, 128)        # [bq, b, ih, j]

    in_maps = []
    for c in range(N_CORES):
        bq, oh = c % B_SPLIT, c // B_SPLIT
        # ur[bq] axes (b, ih, j) -> (j, ih, b) -> [128, FW]
        u0_c = np.ascontiguousarray(
            ur[bq].transpose(2, 1, 0)).reshape(128, FW)
        # [s, o, ih, j] -> (j, s, ih, o): columns kt*128 + o, kt = 2s+ih
        C_c = np.ascontiguousarray(
            M16_r[:, oh].transpose(3, 0, 2, 1)).reshape(128, KT * 128)
        b2_c = np.ascontiguousarray(np.concatenate(
            [bh[oh * O_LOC:(oh + 1) * O_LOC],
             bl[oh * O_LOC:(oh + 1) * O_LOC]]).reshape(1, 256))
        in_maps.append({"u0": u0_c, "C": C_c, "b2": b2_c})

    nc = _get_nc()
    return nc, in_maps


def kernel(x: np.ndarray, breakpoints: np.ndarray, values: np.ndarray,
           **_extra) -> np.ndarray:
    nc, in_maps = prepare(x, breakpoints, values)
    res = run_bass_kernel_spmd(nc, in_maps, list(range(N_CORES)))

    outf = np.empty((B, O), np.float32)
    for c in range(N_CORES):
        bq, oh = c % B_SPLIT, c // B_SPLIT
        outf[bq * B_LOC:(bq + 1) * B_LOC, oh * O_LOC:(oh + 1) * O_LOC] = \
            res.results[c]["out"].T.astype(np.float32)
    return outf


if __name__ == "__main__":
    rng = np.random.default_rng(0)
    x = rng.uniform(-1, 1, (B, I)).astype(np.float32)
    bp = np.tile(np.linspace(-1, 1, S + 1, dtype=np.float32), (O, I, 1))
    v = (rng.standard_normal((O, I, S + 1)) * 0.1).astype(np.float32)
    out = kernel(x, bp, v)
    print("kernel ran, out:", out.shape, out.dtype, float(out.std()))
